# revision 39
# baseline (speedup 1.0000x reference)
"""Trainium2 Bass kernel for nn_BinarizedLinearBlock.

Computes y = clip(BatchNorm1d(x) @ sign(W)^T, -1, 1) for
x [8192, 2048] f32, W [2048, 2048] f32, gamma/beta [2048] f32.

Strategy (8 NeuronCores, data-parallel over batch):
  - Each core gets a batch shard x_j [1024, 2048] and the full weight.
  - BN statistics need the whole batch: each core computes partial
    (sum, sumsq) per feature from its shard, a 16 KB AllReduce combines
    them, then each core normalizes its shard locally.
  - Both matmul operands need the contraction dim (IN) on partitions.
    All transposes run on the DMA xbar (dma_start_transpose): one
    instruction per [128, 2048] fp16 tile produces all 16 transposed
    [128,128] blocks into a 3D destination - zero TensorE time.
  - x: fp32 HWDGE load -> DVE cast fp16 -> xbar transpose -> bn_stats
    -> AllReduce -> per-partition fused normalize (DVE tensor_scalar).
  - W: fp32 HWDGE load (natural row tiles = fat descriptors; column
    tiles degrade to 512B packets) -> ACT Sign (fp32 in, fp16 +-1 out,
    binarize+downcast in one op) -> xbar transpose.
  - Main matmul: lhsT = xn^T tile (fp16), rhs = sign(W)^T (fp16),
    fp32 PSUM accumulation over 16 k-tiles; eviction fuses the
    hardtanh clip via one DVE tensor_scalar (min 1, max -1).
  - The matmul loop runs h-outer: the first output half consumes only
    W o-tiles 0-7, so matmuls start while o-tiles 8-15 still stream.
"""

import sys

sys.path.insert(0, "/opt/trn_rl_repo")

import numpy as np

import concourse.bass as bass
import concourse.bacc as bacc
import concourse.mybir as mybir
import concourse.tile as tile
from concourse.bass_utils import run_bass_kernel_spmd

F32 = mybir.dt.float32
F16 = mybir.dt.float16
ALU = mybir.AluOpType
AFT = mybir.ActivationFunctionType

B, IN, OUT = 8192, 2048, 2048
NCORES = 8
BSH = B // NCORES          # 1024 batch rows per core
KB = BSH // 128            # 8 batch tiles per core
KI = IN // 128             # 16 contraction (input-feature) tiles
KO = OUT // 128            # 16 output-feature (W row) tiles
BN_EPS = 1e-5


def build_kernel_body(tc, y_d, x_d, w_d, gam_d, bet_d):
    nc = tc.nc

    consts = tc.tile_pool(name="consts", bufs=1)
    persist = tc.tile_pool(name="persist", bufs=1)
    xnat_pool = tc.tile_pool(name="xnat", bufs=3)
    xnat16_pool = tc.tile_pool(name="xnat16", bufs=2)
    wstg_pool = tc.tile_pool(name="wstg", bufs=3)
    wsign_pool = tc.tile_pool(name="wsign", bufs=2)
    ysb_pool = tc.tile_pool(name="ysb", bufs=3)
    ypsum = tc.tile_pool(name="ypsum", bufs=3, space="PSUM")
    dram = tc.tile_pool(name="dram", bufs=1, space="DRAM")

    ctxs = [consts, persist, xnat_pool, xnat16_pool, wstg_pool, wsign_pool,
            ysb_pool, ypsum, dram]
    entered = [c.__enter__() for c in ctxs]
    (consts, persist, xnat_pool, xnat16_pool, wstg_pool, wsign_pool,
     ysb_pool, ypsum, dram) = entered

    # ---- constants -------------------------------------------------
    gamma_sb = consts.tile([128, KI], F32)
    beta_sb = consts.tile([128, KI], F32)
    zero_col = consts.tile([128, 1], F32)
    eps_col = consts.tile([128, 1], F32)
    nc.vector.memset(zero_col[:], 0.0)
    nc.vector.memset(eps_col[:], BN_EPS)
    nc.scalar.dma_start(gamma_sb[:], gam_d[:, :])
    nc.scalar.dma_start(beta_sb[:], bet_d[:, :])

    # ---- persistent SBUF tensors ----------------------------------
    xT3 = persist.tile([128, KI, BSH], F16)       # x^T, later xn^T in place
    wbT3 = persist.tile([128, KI, OUT], F16)      # sign(W)^T, exact +-1

    # ---- Phase X: load x fp32, cast fp16 on DVE, xbar transpose ---
    bnst = persist.tile([128, KI, 2, 6], F32)
    bnag = persist.tile([128, KI, 2], F32)
    for b in range(KB):
        xnat = xnat_pool.tile([128, IN], F32)
        # alternate the two HWDGE rings; x sits ahead of W in both
        # FIFOs so it gets the full HBM bandwidth first
        nc.scalar.dma_start(xnat[:], x_d[b * 128:(b + 1) * 128, :])
        xnat16 = xnat16_pool.tile([128, IN], F16)
        nc.vector.tensor_copy(xnat16[:], xnat[:])
        # transposes ride the sync ring EXCLUSIVELY: concurrent
        # transpose + copy traffic on one ring corrupts data (xbar
        # mode-switch hazard); bulk copies go on the scalar ring
        nc.sync.dma_start_transpose(xT3[:, :, b * 128:(b + 1) * 128], xnat16[:])
        if b in (KB // 2 - 1, KB - 1):
            # bn_stats in two half-batch sweeps (walrus requires
            # exactly 6 output elems/partition per op)
            ch = 0 if b == KB // 2 - 1 else 1
            for t in range(KI):
                nc.vector.bn_stats(
                    bnst[:, t, ch, :], xT3[:, t, ch * 512:(ch + 1) * 512]
                )
    for t in range(KI):
        nc.vector.bn_aggr(bnag[:, t, :], bnst[:, t, :, :])

    # local sums for the AllReduce: s1 = mean * BSH ; s2 = (var + mean^2) * BSH
    stats = persist.tile([128, 2 * KI], F32)
    means = bnag[:, :, 0]
    vars_ = bnag[:, :, 1]
    msq = persist.tile([128, KI], F32)
    nc.vector.tensor_scalar(stats[:, 0:KI], means, float(BSH), None, op0=ALU.mult)
    nc.vector.tensor_tensor(msq[:], means, means, op=ALU.mult)
    nc.vector.tensor_tensor(msq[:], vars_, msq[:], op=ALU.add)
    nc.vector.tensor_scalar(stats[:, KI:2 * KI], msq[:], float(BSH), None, op0=ALU.mult)

    # ---- Phase R: AllReduce over the 8 cores ----------------------
    # Bounce DMAs ride the (otherwise empty) SWDGE queue so they are
    # not FIFO-stuck behind the bulk x/W loads on the HWDGE rings.
    cc_in = dram.tile([128, 2 * KI], F32)
    cc_out = dram.tile([128, 2 * KI], F32)
    nc.gpsimd.dma_start(cc_in[:], stats[:])
    nc.gpsimd.collective_compute(
        "AllReduce",
        ALU.add,
        replica_groups=[list(range(NCORES))],
        ins=[cc_in[:].opt()],
        outs=[cc_out[:].opt()],
    )

    # ---- Phase W: load W row-tiles fp32 (natural layout = fat 8 KB
    # DMA segments), binarize+downcast with one ACT Sign (fp32 in,
    # fp16 +-1 out - reading fp32 keeps tiny weights' signs exact),
    # then one xbar transpose per o-tile.
    def w_prep(o):
        wstg = wstg_pool.tile([128, IN], F32, name=f"wstg{o}", tag="wstg")
        nc.scalar.dma_start(wstg[:], w_d[o * 128:(o + 1) * 128, :])
        wsg = wsign_pool.tile([128, IN], F16, name=f"wsign{o}", tag="wsign")
        nc.scalar.sign(wsg[:], wstg[:], bias=zero_col[:])
        nc.sync.dma_start_transpose(wbT3[:, :, o * 128:(o + 1) * 128], wsg[:])

    # o-tiles 0-7 cover the h=0 half of the output: prepare them first;
    # the ACT-stream order is [sign o0-7, sqrt (lands right when the
    # AllReduce completes), sign o8-15] so no stream blocks another.
    for o in range(KO // 2):
        w_prep(o)

    gstats = persist.tile([128, 2 * KI], F32)
    nc.gpsimd.dma_start(gstats[:], cc_out[:])

    # ---- Phase N: compute a, c and normalize x^T in place ---------
    # a = gamma * rsqrt(var + eps);  c = beta - mean * a
    meang = persist.tile([128, KI], F32)
    ex2g = persist.tile([128, KI], F32)
    varg = persist.tile([128, KI], F32)
    stdg = persist.tile([128, KI], F32)
    invg = persist.tile([128, KI], F32)
    a_sc = persist.tile([128, KI], F32)
    c_sc = persist.tile([128, KI], F32)
    nc.vector.tensor_scalar(meang[:], gstats[:, 0:KI], 1.0 / B, None, op0=ALU.mult)
    nc.vector.tensor_scalar(ex2g[:], gstats[:, KI:2 * KI], 1.0 / B, None, op0=ALU.mult)
    nc.vector.tensor_tensor(varg[:], meang[:], meang[:], op=ALU.mult)
    nc.vector.tensor_tensor(varg[:], ex2g[:], varg[:], op=ALU.subtract)
    nc.scalar.activation(stdg[:], varg[:], AFT.Sqrt, bias=eps_col[:])
    nc.vector.reciprocal(invg[:], stdg[:])
    nc.vector.tensor_tensor(a_sc[:], gamma_sb[:], invg[:], op=ALU.mult)
    nc.vector.tensor_tensor(c_sc[:], meang[:], a_sc[:], op=ALU.mult)
    nc.vector.tensor_tensor(c_sc[:], beta_sb[:], c_sc[:], op=ALU.subtract)

    for t in range(KI):
        nc.vector.tensor_scalar(
            xT3[:, t, :], xT3[:, t, :],
            a_sc[:, t:t + 1], c_sc[:, t:t + 1],
            op0=ALU.mult, op1=ALU.add,
        )

    for o in range(KO // 2, KO):
        w_prep(o)

    # ---- Phase M: main matmul + fused clip eviction ---------------
    # h-outer: the h=0 half consumes only o-tiles 0-7, so it starts as
    # soon as the AllReduce lands while o-tiles 8-15 still stream in.
    for h in range(2):
        for b in range(KB):
            yp = ypsum.tile([128, 1024], F32)
            for t in range(KI):
                lhs = xT3[:, t, b * 128:(b + 1) * 128]
                for n2 in range(2):
                    nc.tensor.matmul(
                        yp[:, n2 * 512:(n2 + 1) * 512],
                        lhs,
                        wbT3[:, t, h * 1024 + n2 * 512: h * 1024 + (n2 + 1) * 512],
                        start=(t == 0),
                        stop=(t == KI - 1),
                    )
            ysb = ysb_pool.tile([128, 1024], F32)
            nc.vector.tensor_scalar(
                ysb[:], yp[:], 1.0, -1.0, op0=ALU.min, op1=ALU.max
            )
            nc.scalar.dma_start(
                y_d[b * 128:(b + 1) * 128, h * 1024:(h + 1) * 1024], ysb[:]
            )

    for c in reversed(ctxs):
        c.__exit__(None, None, None)


def build_program():
    nc = bacc.Bacc(
        "TRN2",
        target_bir_lowering=False,
        debug=False,
        num_devices=NCORES,
    )
    x_d = nc.dram_tensor("x", [BSH, IN], F32, kind="ExternalInput")
    w_d = nc.dram_tensor("weight", [OUT, IN], F32, kind="ExternalInput")
    gam_d = nc.dram_tensor("gamma_blk", [128, KI], F32, kind="ExternalInput")
    bet_d = nc.dram_tensor("beta_blk", [128, KI], F32, kind="ExternalInput")
    y_d = nc.dram_tensor("y", [BSH, OUT], F32, kind="ExternalOutput")

    with tile.TileContext(nc) as tc:
        build_kernel_body(
            tc, y_d[:, :], x_d[:, :], w_d[:, :], gam_d[:, :], bet_d[:, :],
        )
    nc.compile()
    return nc


_CACHE = {}


def _get_program():
    if "nc" not in _CACHE:
        _CACHE["nc"] = build_program()
    return _CACHE["nc"]


def make_in_maps(x, weight, gamma, beta):
    x = np.ascontiguousarray(np.asarray(x, dtype=np.float32))
    weight = np.ascontiguousarray(np.asarray(weight, dtype=np.float32))
    gamma = np.asarray(gamma, dtype=np.float32)
    beta = np.asarray(beta, dtype=np.float32)
    gamma_blk = np.ascontiguousarray(gamma.reshape(KI, 128).T)
    beta_blk = np.ascontiguousarray(beta.reshape(KI, 128).T)
    in_maps = []
    for j in range(NCORES):
        in_maps.append({
            "x": np.ascontiguousarray(x[j * BSH:(j + 1) * BSH]),
            "weight": weight,
            "gamma_blk": gamma_blk,
            "beta_blk": beta_blk,
        })
    return in_maps


def run(x, weight, gamma, beta, **spmd_kwargs):
    """Run on hardware; returns (y_full, BassKernelResults)."""
    nc = _get_program()
    in_maps = make_in_maps(x, weight, gamma, beta)
    res = run_bass_kernel_spmd(nc, in_maps, core_ids=list(range(NCORES)), **spmd_kwargs)
    y = np.concatenate([r["y"] for r in res.results], axis=0)
    return np.asarray(y, dtype=np.float32), res


def run_traced(x, weight, gamma, beta, profile_dir=None):
    """Run with NTFF capture via the axon sidechannel; returns
    (y_full, per_core_exec_ns, profile_dir)."""
    import ctypes, glob, tempfile
    from concourse import bass2jax
    import gauge.profiler
    from concourse._compat import FishPath

    nc = _get_program()
    in_maps = make_in_maps(x, weight, gamma, beta)

    lib = ctypes.CDLL("/opt/axon/libaxon_pjrt.so")
    lib.axon_start_nrt_profile.argtypes = [
        ctypes.POINTER(ctypes.c_int64), ctypes.c_size_t]
    lib.axon_start_nrt_profile.restype = ctypes.c_int64
    lib.axon_stop_nrt_profile.argtypes = [ctypes.c_char_p]
    lib.axon_stop_nrt_profile.restype = ctypes.c_int64

    if profile_dir is None:
        profile_dir = tempfile.mkdtemp(prefix="ntff_")
    rc = lib.axon_start_nrt_profile(None, 0)
    assert rc == 0, f"axon_start_nrt_profile rc={rc}"
    try:
        results = bass2jax.run_bass_via_pjrt(nc, in_maps, n_cores=NCORES)
    finally:
        n = lib.axon_stop_nrt_profile(profile_dir.encode())
    y = np.concatenate([r["y"] for r in results], axis=0)
    if n <= 0:
        return np.asarray(y, dtype=np.float32), None, profile_dir

    profile = gauge.profiler.Profile(
        profile_path=FishPath(profile_dir),
        kernel_dev_mode=True,
        profile_on_exit=False,
        bass_kernel=nc.m,
        offline_processing=True,
        fname="*_body*",
    )
    perfetto_results = profile.to_perfetto(model_index=tuple(range(NCORES)))
    exec_ns = {}
    for i, pr in enumerate(perfetto_results or []):
        exec_ns[i] = pr.exec_time_ns
    return np.asarray(y, dtype=np.float32), exec_ns, profile_dir


def kernel(x, weight, gamma, beta):
    y, _ = run(x, weight, gamma, beta)
    return y


# revision 42
# speedup vs baseline: 1.0519x; 1.0519x over previous
"""Trainium2 Bass kernel for nn_BinarizedLinearBlock.

Computes y = clip(BatchNorm1d(x) @ sign(W)^T, -1, 1) for
x [8192, 2048] f32, W [2048, 2048] f32, gamma/beta [2048] f32.

Strategy (8 NeuronCores, data-parallel over batch):
  - Each core gets a batch shard x_j [1024, 2048] and the full weight.
  - BN statistics need the whole batch: each core computes partial
    (sum, sumsq) per feature from its shard, a 16 KB AllReduce combines
    them, then each core normalizes its shard locally.
  - Both matmul operands need the contraction dim (IN) on partitions.
    All transposes run on the DMA xbar (dma_start_transpose): one
    instruction per [128, 2048] fp16 tile produces all 16 transposed
    [128,128] blocks - zero TensorE time.  Transposed tensors use
    b-major / o-major layouts so the xbar destination is CONTIGUOUS
    (a strided destination scatters 256 B packets and runs ~8x slower).
  - Ring discipline: transposes ride the sync HWDGE ring exclusively
    (interleaving transpose + copy traffic on one ring corrupts data);
    bulk copies ride the scalar ring, except the x loads which split
    across both (the sync ring is clean until the first transpose,
    which is held behind the last x load by an explicit dep).
  - x: fp32 load -> DVE cast fp16 -> xbar transpose -> sum (DVE
    strided reduce) + sumsq (ACT Square with accumulate) -> AllReduce
    -> per-partition fused normalize (DVE tensor_scalar, in place).
  - W: fp32 load (natural row tiles = fat descriptors) -> ACT Sign
    (fp32 in, fp16 +-1 out, binarize+downcast in one op) -> xbar.
  - Main matmul: lhsT = xn^T tile (fp16), rhs = sign(W)^T (fp16, 3D
    o-major view), fp32 PSUM accumulation over 16 k-tiles; eviction
    fuses the hardtanh clip via one DVE tensor_scalar (min 1, max -1).
  - The matmul loop runs h-outer: the first output half consumes only
    W o-tiles 0-7, so matmuls start while o-tiles 8-15 still stream.
"""

import sys

sys.path.insert(0, "/opt/trn_rl_repo")

import numpy as np

import concourse.bass as bass
import concourse.bacc as bacc
import concourse.mybir as mybir
import concourse.tile as tile
from concourse.bass_utils import run_bass_kernel_spmd

F32 = mybir.dt.float32
F16 = mybir.dt.float16
ALU = mybir.AluOpType
AFT = mybir.ActivationFunctionType
AXL = mybir.AxisListType

B, IN, OUT = 8192, 2048, 2048
NCORES = 8
BSH = B // NCORES          # 1024 batch rows per core
KB = BSH // 128            # 8 batch tiles per core
KI = IN // 128             # 16 contraction (input-feature) tiles
KO = OUT // 128            # 16 output-feature (W row) tiles
BN_EPS = 1e-5


def build_kernel_body(tc, y_d, x_d, w_d, gam_d, bet_d):
    nc = tc.nc

    consts = tc.tile_pool(name="consts", bufs=1)
    persist = tc.tile_pool(name="persist", bufs=1)
    xnat_pool = tc.tile_pool(name="xnat", bufs=2)
    # bufs=KB: xbar transposes are held behind the last sync-ring x
    # load (ring separation), so all 8 fp16 tiles must coexist or the
    # slot-reuse chain deadlocks against that dependency
    xnat16_pool = tc.tile_pool(name="xnat16", bufs=KB)
    wstg_pool = tc.tile_pool(name="wstg", bufs=2)
    wsign_pool = tc.tile_pool(name="wsign", bufs=2)
    sq_pool = tc.tile_pool(name="sqscratch", bufs=2)
    ysb_pool = tc.tile_pool(name="ysb", bufs=3)
    ypsum = tc.tile_pool(name="ypsum", bufs=3, space="PSUM")
    dram = tc.tile_pool(name="dram", bufs=1, space="DRAM")

    ctxs = [consts, persist, xnat_pool, xnat16_pool, wstg_pool, wsign_pool,
            sq_pool, ysb_pool, ypsum, dram]
    entered = [c.__enter__() for c in ctxs]
    (consts, persist, xnat_pool, xnat16_pool, wstg_pool, wsign_pool,
     sq_pool, ysb_pool, ypsum, dram) = entered

    # ---- constants -------------------------------------------------
    gamma_sb = consts.tile([128, KI], F32)
    beta_sb = consts.tile([128, KI], F32)
    zero_col = consts.tile([128, 1], F32)
    eps_col = consts.tile([128, 1], F32)
    nc.vector.memset(zero_col[:], 0.0)
    nc.vector.memset(eps_col[:], BN_EPS)
    nc.scalar.dma_start(gamma_sb[:], gam_d[:, :])
    nc.scalar.dma_start(beta_sb[:], bet_d[:, :])

    # ---- persistent SBUF tensors (b-major / o-major so the xbar
    # transpose destination [:, j, :, :] is contiguous) --------------
    xT4 = persist.tile([128, KB, KI, 128], F16)   # [i_loc, b_tile, t, b_loc]
    wbT4 = persist.tile([128, KO, KI, 128], F16)  # [i_loc, o_tile, t, o_loc]

    # ---- Phase X: load x fp32 (both rings), cast fp16 on DVE, -----
    # xbar transpose (sync ring, held behind the last x load)
    x_loads = []
    x_xbars = []
    for b in range(KB):
        xnat = xnat_pool.tile([128, IN], F32)
        eng = nc.sync if b % 2 == 0 else nc.scalar
        x_loads.append(eng.dma_start(xnat[:], x_d[b * 128:(b + 1) * 128, :]))
        xnat16 = xnat16_pool.tile([128, IN], F16)
        nc.vector.tensor_copy(xnat16[:], xnat[:])
        x_xbars.append(
            nc.sync.dma_start_transpose(xT4[:, b, :, :], xnat16[:])
        )
    # keep the sync ring free of copy traffic while transposes run
    for xb in x_xbars:
        tile.add_dep_helper(xb.ins, x_loads[-2].ins,
                            reason="xbar ring separation")

    # ---- Phase S: per-feature sum & sumsq -------------------------
    # sum: one strided DVE reduce per t over [128, KB, 128]
    # sumsq: one ACT Square with free-dim accumulate per t
    stats = persist.tile([128, 2 * KI], F32)
    for t in range(KI):
        nc.vector.tensor_reduce(
            stats[:, t:t + 1], xT4[:, :, t, :], axis=AXL.XY, op=ALU.add
        )
        sqs = sq_pool.tile([128, KB, 128], F16)
        nc.scalar.activation(
            sqs[:], xT4[:, :, t, :], AFT.Square,
            bias=zero_col[:], accum_out=stats[:, KI + t:KI + t + 1],
        )

    # ---- Phase R: AllReduce over the 8 cores ----------------------
    # Bounce DMAs ride the (otherwise empty) SWDGE queue so they are
    # not FIFO-stuck behind the bulk x/W loads on the HWDGE rings.
    cc_in = dram.tile([128, 2 * KI], F32)
    cc_out = dram.tile([128, 2 * KI], F32)
    nc.gpsimd.dma_start(cc_in[:], stats[:])
    nc.gpsimd.collective_compute(
        "AllReduce",
        ALU.add,
        replica_groups=[list(range(NCORES))],
        ins=[cc_in[:].opt()],
        outs=[cc_out[:].opt()],
    )

    # ---- Phase W: load W row-tiles fp32 on the scalar ring
    # (natural layout = fat 8 KB DMA segments), binarize+downcast with
    # one ACT Sign (fp32 in, fp16 +-1 out - reading fp32 keeps tiny
    # weights' signs exact), then one xbar transpose per o-tile on the
    # sync ring.
    def w_prep(o):
        wstg = wstg_pool.tile([128, IN], F32, name=f"wstg{o}", tag="wstg")
        nc.scalar.dma_start(wstg[:], w_d[o * 128:(o + 1) * 128, :])
        wsg = wsign_pool.tile([128, IN], F16, name=f"wsign{o}", tag="wsign")
        nc.scalar.sign(wsg[:], wstg[:], bias=zero_col[:])
        nc.sync.dma_start_transpose(wbT4[:, o, :, :], wsg[:])

    # o-tiles 0-7 cover the h=0 half of the output: prepare them first;
    # the ACT-stream order is [sq-stats, sign o0-7, sqrt, sign o8-15].
    for o in range(KO // 2):
        w_prep(o)

    gstats = persist.tile([128, 2 * KI], F32)
    nc.gpsimd.dma_start(gstats[:], cc_out[:])

    # ---- Phase N: compute a, c and normalize x^T in place ---------
    # a = gamma * rsqrt(var + eps);  c = beta - mean * a
    meang = persist.tile([128, KI], F32)
    ex2g = persist.tile([128, KI], F32)
    varg = persist.tile([128, KI], F32)
    stdg = persist.tile([128, KI], F32)
    invg = persist.tile([128, KI], F32)
    a_sc = persist.tile([128, KI], F32)
    c_sc = persist.tile([128, KI], F32)
    nc.vector.tensor_scalar(meang[:], gstats[:, 0:KI], 1.0 / B, None, op0=ALU.mult)
    nc.vector.tensor_scalar(ex2g[:], gstats[:, KI:2 * KI], 1.0 / B, None, op0=ALU.mult)
    nc.vector.tensor_tensor(varg[:], meang[:], meang[:], op=ALU.mult)
    nc.vector.tensor_tensor(varg[:], ex2g[:], varg[:], op=ALU.subtract)
    nc.scalar.activation(stdg[:], varg[:], AFT.Sqrt, bias=eps_col[:])
    nc.vector.reciprocal(invg[:], stdg[:])
    nc.vector.tensor_tensor(a_sc[:], gamma_sb[:], invg[:], op=ALU.mult)
    nc.vector.tensor_tensor(c_sc[:], meang[:], a_sc[:], op=ALU.mult)
    nc.vector.tensor_tensor(c_sc[:], beta_sb[:], c_sc[:], op=ALU.subtract)

    for t in range(KI):
        nc.vector.tensor_scalar(
            xT4[:, :, t, :], xT4[:, :, t, :],
            a_sc[:, t:t + 1], c_sc[:, t:t + 1],
            op0=ALU.mult, op1=ALU.add,
        )

    for o in range(KO // 2, KO):
        w_prep(o)

    # ---- Phase M: main matmul + fused clip eviction ---------------
    # h-outer: the h=0 half consumes only o-tiles 0-7, so it starts as
    # soon as the AllReduce lands while o-tiles 8-15 still stream in.
    for h in range(2):
        for b in range(KB):
            yp = ypsum.tile([128, 1024], F32)
            for t in range(KI):
                lhs = xT4[:, b, t, :]
                for n2 in range(2):
                    og = 4 * (2 * h + n2)
                    nc.tensor.matmul(
                        yp[:, n2 * 512:(n2 + 1) * 512],
                        lhs,
                        wbT4[:, og:og + 4, t, :],
                        start=(t == 0),
                        stop=(t == KI - 1),
                    )
            ysb = ysb_pool.tile([128, 1024], F32)
            nc.vector.tensor_scalar(
                ysb[:], yp[:], 1.0, -1.0, op0=ALU.min, op1=ALU.max
            )
            nc.scalar.dma_start(
                y_d[b * 128:(b + 1) * 128, h * 1024:(h + 1) * 1024], ysb[:]
            )

    for c in reversed(ctxs):
        c.__exit__(None, None, None)


def build_program():
    nc = bacc.Bacc(
        "TRN2",
        target_bir_lowering=False,
        debug=False,
        num_devices=NCORES,
    )
    x_d = nc.dram_tensor("x", [BSH, IN], F32, kind="ExternalInput")
    w_d = nc.dram_tensor("weight", [OUT, IN], F32, kind="ExternalInput")
    gam_d = nc.dram_tensor("gamma_blk", [128, KI], F32, kind="ExternalInput")
    bet_d = nc.dram_tensor("beta_blk", [128, KI], F32, kind="ExternalInput")
    y_d = nc.dram_tensor("y", [BSH, OUT], F32, kind="ExternalOutput")

    with tile.TileContext(nc) as tc:
        build_kernel_body(
            tc, y_d[:, :], x_d[:, :], w_d[:, :], gam_d[:, :], bet_d[:, :],
        )
    nc.compile()
    return nc


_CACHE = {}


def _get_program():
    if "nc" not in _CACHE:
        _CACHE["nc"] = build_program()
    return _CACHE["nc"]


def make_in_maps(x, weight, gamma, beta):
    x = np.ascontiguousarray(np.asarray(x, dtype=np.float32))
    weight = np.ascontiguousarray(np.asarray(weight, dtype=np.float32))
    gamma = np.asarray(gamma, dtype=np.float32)
    beta = np.asarray(beta, dtype=np.float32)
    gamma_blk = np.ascontiguousarray(gamma.reshape(KI, 128).T)
    beta_blk = np.ascontiguousarray(beta.reshape(KI, 128).T)
    in_maps = []
    for j in range(NCORES):
        in_maps.append({
            "x": np.ascontiguousarray(x[j * BSH:(j + 1) * BSH]),
            "weight": weight,
            "gamma_blk": gamma_blk,
            "beta_blk": beta_blk,
        })
    return in_maps


def run(x, weight, gamma, beta, **spmd_kwargs):
    """Run on hardware; returns (y_full, BassKernelResults)."""
    nc = _get_program()
    in_maps = make_in_maps(x, weight, gamma, beta)
    res = run_bass_kernel_spmd(nc, in_maps, core_ids=list(range(NCORES)), **spmd_kwargs)
    y = np.concatenate([r["y"] for r in res.results], axis=0)
    return np.asarray(y, dtype=np.float32), res


def run_traced(x, weight, gamma, beta, profile_dir=None):
    """Run with NTFF capture via the axon sidechannel; returns
    (y_full, per_core_exec_ns, profile_dir)."""
    import ctypes, tempfile
    from concourse import bass2jax
    import gauge.profiler
    from concourse._compat import FishPath

    nc = _get_program()
    in_maps = make_in_maps(x, weight, gamma, beta)

    lib = ctypes.CDLL("/opt/axon/libaxon_pjrt.so")
    lib.axon_start_nrt_profile.argtypes = [
        ctypes.POINTER(ctypes.c_int64), ctypes.c_size_t]
    lib.axon_start_nrt_profile.restype = ctypes.c_int64
    lib.axon_stop_nrt_profile.argtypes = [ctypes.c_char_p]
    lib.axon_stop_nrt_profile.restype = ctypes.c_int64

    if profile_dir is None:
        profile_dir = tempfile.mkdtemp(prefix="ntff_")
    rc = lib.axon_start_nrt_profile(None, 0)
    assert rc == 0, f"axon_start_nrt_profile rc={rc}"
    try:
        results = bass2jax.run_bass_via_pjrt(nc, in_maps, n_cores=NCORES)
    finally:
        n = lib.axon_stop_nrt_profile(profile_dir.encode())
    y = np.concatenate([r["y"] for r in results], axis=0)
    if n <= 0:
        return np.asarray(y, dtype=np.float32), None, profile_dir

    profile = gauge.profiler.Profile(
        profile_path=FishPath(profile_dir),
        kernel_dev_mode=True,
        profile_on_exit=False,
        bass_kernel=nc.m,
        offline_processing=True,
        fname="*_body*",
    )
    perfetto_results = profile.to_perfetto(model_index=tuple(range(NCORES)))
    exec_ns = {}
    for i, pr in enumerate(perfetto_results or []):
        exec_ns[i] = pr.exec_time_ns
    return np.asarray(y, dtype=np.float32), exec_ns, profile_dir


def kernel(x, weight, gamma, beta):
    y, _ = run(x, weight, gamma, beta)
    return y


# revision 43
# speedup vs baseline: 1.3485x; 1.2820x over previous
"""Trainium2 Bass kernel for nn_BinarizedLinearBlock.

Computes y = clip(BatchNorm1d(x) @ sign(W)^T, -1, 1) for
x [8192, 2048] f32, W [2048, 2048] f32, gamma/beta [2048] f32.

Strategy (8 NeuronCores, data-parallel over batch):
  - Each core gets a batch shard x_j [1024, 2048] and the full weight.
  - BN statistics need the whole batch: each core computes partial
    (sum, sumsq) per feature from its shard, a 16 KB AllReduce combines
    them, then each core normalizes its shard locally.
  - Both matmul operands need the contraction dim (IN) on partitions,
    so x and W are transposed on the PE (128x128 identity matmuls,
    4 per PSUM bank).  16-bit transposes: fp32 inputs are pre-cast on
    DVE (GpSimd casts are 7 us/tile, DVE ~1 us; fp32 PE transposes pay
    2 cycles/row).
  - Loads are plain fp32 on both HWDGE rings (SWDGE dtype-cast DMAs
    collapse to ~115 GB/s single-queue; W column-slices degrade to
    512 B packets, so W loads natural row-tiles).  x is queued ahead
    of W in both ring FIFOs; y stores follow on the sync ring.
  - W path: DVE cast fp32->bf16 (keeps fp32 exponent range so sign()
    is never corrupted by underflow), PE transpose, ACT Sign on the
    PSUM eviction emits exact +-1 in fp16.
  - The AllReduce bounce rides the otherwise-empty SWDGE queue.  The
    single ACT Sqrt is emitted between the two W sign batches so it
    never blocks the ACT queue.
  - Main matmul: lhsT = xn^T tile (fp16), rhs = sign(W)^T (fp16),
    fp32 PSUM accumulation over 16 k-tiles; eviction fuses the
    hardtanh clip via one DVE tensor_scalar (min 1, max -1).
  - h-outer matmul loop: the first output half consumes only W
    o-tiles 0-7, so matmuls start while o-tiles 8-15 still stream.
"""

import sys

sys.path.insert(0, "/opt/trn_rl_repo")

import numpy as np
import ml_dtypes

import concourse.bass as bass
import concourse.bacc as bacc
import concourse.mybir as mybir
import concourse.tile as tile
from concourse.bass_utils import run_bass_kernel_spmd

F32 = mybir.dt.float32
F16 = mybir.dt.float16
BF16 = mybir.dt.bfloat16
ALU = mybir.AluOpType
AFT = mybir.ActivationFunctionType

B, IN, OUT = 8192, 2048, 2048
NCORES = 8
BSH = B // NCORES          # 1024 batch rows per core
KB = BSH // 128            # 8 batch tiles per core
KI = IN // 128             # 16 contraction (input-feature) tiles
KO = OUT // 128            # 16 output-feature (W row) tiles
BN_EPS = 1e-5


def build_kernel_body(tc, y_d, x_d, w_d, gam_d, bet_d, idf_d, idb_d):
    nc = tc.nc

    consts = tc.tile_pool(name="consts", bufs=1)
    persist = tc.tile_pool(name="persist", bufs=1)
    xnat_pool = tc.tile_pool(name="xnat", bufs=3)
    xnat16_pool = tc.tile_pool(name="xnat16", bufs=3)
    wstg_pool = tc.tile_pool(name="wstg", bufs=3)
    wstg16_pool = tc.tile_pool(name="wstg16", bufs=3)
    ysb_pool = tc.tile_pool(name="ysb", bufs=3)
    tpsum = tc.tile_pool(name="tpsum", bufs=2, space="PSUM")
    ypsum = tc.tile_pool(name="ypsum", bufs=2, space="PSUM")
    dram = tc.tile_pool(name="dram", bufs=1, space="DRAM")

    ctxs = [consts, persist, xnat_pool, xnat16_pool, wstg_pool, wstg16_pool,
            ysb_pool, tpsum, ypsum, dram]
    entered = [c.__enter__() for c in ctxs]
    (consts, persist, xnat_pool, xnat16_pool, wstg_pool, wstg16_pool,
     ysb_pool, tpsum, ypsum, dram) = entered

    # ---- constants -------------------------------------------------
    ident_f = consts.tile([128, 128], F16)
    ident_b = consts.tile([128, 128], BF16)
    gamma_sb = consts.tile([128, KI], F32)
    beta_sb = consts.tile([128, KI], F32)
    zero_col = consts.tile([128, 1], F32)
    eps_col = consts.tile([128, 1], F32)
    nc.vector.memset(zero_col[:], 0.0)
    nc.vector.memset(eps_col[:], BN_EPS)
    nc.scalar.dma_start(ident_f[:], idf_d[:, :])
    nc.scalar.dma_start(ident_b[:], idb_d[:, :])
    nc.scalar.dma_start(gamma_sb[:], gam_d[:, :])
    nc.scalar.dma_start(beta_sb[:], bet_d[:, :])

    # ---- persistent SBUF tensors ----------------------------------
    xT3 = persist.tile([128, KI, BSH], F16)       # x^T, later xn^T in place
    wbT3 = persist.tile([128, KI, OUT], F16)      # sign(W)^T, exact +-1

    # ---- Phase X: load x fp32 (both rings), DVE cast fp16, --------
    # PE transpose (4 per PSUM bank), DVE evict, bn_stats half-sweeps
    bnst = persist.tile([128, KI, 2, 6], F32)
    bnag = persist.tile([128, KI, 2], F32)
    for b in range(KB):
        xnat = xnat_pool.tile([128, IN], F32)
        eng = nc.sync if b % 2 == 0 else nc.scalar
        eng.dma_start(xnat[:], x_d[b * 128:(b + 1) * 128, :])
        xnat16 = xnat16_pool.tile([128, IN], F16)
        nc.vector.tensor_copy(xnat16[:], xnat[:])
        for tg in range(KI // 4):
            t = tg * 4
            ps = tpsum.tile([128, 4, 128], F16, tag="xT")
            for j in range(4):
                nc.tensor.transpose(
                    ps[:, j, :], xnat16[:, (t + j) * 128:(t + j + 1) * 128],
                    ident_f[:]
                )
            nc.vector.tensor_copy(xT3[:, t:t + 4, b * 128:(b + 1) * 128], ps[:])
        if b in (KB // 2 - 1, KB - 1):
            ch = 0 if b == KB // 2 - 1 else 1
            for t in range(KI):
                nc.vector.bn_stats(
                    bnst[:, t, ch, :], xT3[:, t, ch * 512:(ch + 1) * 512]
                )
    for t in range(KI):
        nc.vector.bn_aggr(bnag[:, t, :], bnst[:, t, :, :])

    # local sums for the AllReduce: s1 = mean * BSH ; s2 = (var + mean^2) * BSH
    stats = persist.tile([128, 2 * KI], F32)
    means = bnag[:, :, 0]
    vars_ = bnag[:, :, 1]
    msq = persist.tile([128, KI], F32)
    nc.vector.tensor_scalar(stats[:, 0:KI], means, float(BSH), None, op0=ALU.mult)
    nc.vector.tensor_tensor(msq[:], means, means, op=ALU.mult)
    nc.vector.tensor_tensor(msq[:], vars_, msq[:], op=ALU.add)
    nc.vector.tensor_scalar(stats[:, KI:2 * KI], msq[:], float(BSH), None, op0=ALU.mult)

    # ---- Phase R: AllReduce over the 8 cores (SWDGE bounce) -------
    cc_in = dram.tile([128, 2 * KI], F32)
    cc_out = dram.tile([128, 2 * KI], F32)
    nc.gpsimd.dma_start(cc_in[:], stats[:])
    nc.gpsimd.collective_compute(
        "AllReduce",
        ALU.add,
        replica_groups=[list(range(NCORES))],
        ins=[cc_in[:].opt()],
        outs=[cc_out[:].opt()],
    )

    # ---- Phase W: natural fp32 row-tiles on both rings (queued ----
    # after x), DVE cast to bf16, PE transpose, ACT Sign eviction.
    def w_prep(o):
        wstg = wstg_pool.tile([128, IN], F32, name=f"wstg{o}", tag="wstg")
        eng = nc.sync if o % 2 == 0 else nc.scalar
        eng.dma_start(wstg[:], w_d[o * 128:(o + 1) * 128, :])
        wstg16 = wstg16_pool.tile([128, IN], BF16, name=f"wstg16_{o}", tag="wstg16")
        nc.vector.tensor_copy(wstg16[:], wstg[:])
        for tg in range(KI // 4):
            t = tg * 4
            ps = tpsum.tile([128, 4, 128], BF16, tag="wT", name=f"psw{o}_{tg}")
            for j in range(4):
                nc.tensor.transpose(
                    ps[:, j, :], wstg16[:, (t + j) * 128:(t + j + 1) * 128],
                    ident_b[:]
                )
            nc.scalar.sign(
                wbT3[:, t:t + 4, o * 128:(o + 1) * 128],
                ps[:],
                bias=zero_col[:],
            )

    for o in range(KO // 2):
        w_prep(o)

    gstats = persist.tile([128, 2 * KI], F32)
    nc.gpsimd.dma_start(gstats[:], cc_out[:])

    # ---- Phase N: a = gamma * rsqrt(var+eps); c = beta - mean * a -
    # (the lone ACT Sqrt sits between the two sign batches, so it
    # waits for the AllReduce without blocking any sign eviction)
    meang = persist.tile([128, KI], F32)
    ex2g = persist.tile([128, KI], F32)
    varg = persist.tile([128, KI], F32)
    stdg = persist.tile([128, KI], F32)
    invg = persist.tile([128, KI], F32)
    a_sc = persist.tile([128, KI], F32)
    c_sc = persist.tile([128, KI], F32)
    nc.vector.tensor_scalar(meang[:], gstats[:, 0:KI], 1.0 / B, None, op0=ALU.mult)
    nc.vector.tensor_scalar(ex2g[:], gstats[:, KI:2 * KI], 1.0 / B, None, op0=ALU.mult)
    nc.vector.tensor_tensor(varg[:], meang[:], meang[:], op=ALU.mult)
    nc.vector.tensor_tensor(varg[:], ex2g[:], varg[:], op=ALU.subtract)
    nc.scalar.activation(stdg[:], varg[:], AFT.Sqrt, bias=eps_col[:])
    nc.vector.reciprocal(invg[:], stdg[:])
    nc.vector.tensor_tensor(a_sc[:], gamma_sb[:], invg[:], op=ALU.mult)
    nc.vector.tensor_tensor(c_sc[:], meang[:], a_sc[:], op=ALU.mult)
    nc.vector.tensor_tensor(c_sc[:], beta_sb[:], c_sc[:], op=ALU.subtract)

    for t in range(KI):
        nc.vector.tensor_scalar(
            xT3[:, t, :], xT3[:, t, :],
            a_sc[:, t:t + 1], c_sc[:, t:t + 1],
            op0=ALU.mult, op1=ALU.add,
        )

    for o in range(KO // 2, KO):
        w_prep(o)

    # ---- Phase M: main matmul + fused clip eviction ---------------
    for h in range(2):
        for b in range(KB):
            yp = ypsum.tile([128, 1024], F32)
            for t in range(KI):
                lhs = xT3[:, t, b * 128:(b + 1) * 128]
                for n2 in range(2):
                    nc.tensor.matmul(
                        yp[:, n2 * 512:(n2 + 1) * 512],
                        lhs,
                        wbT3[:, t, h * 1024 + n2 * 512: h * 1024 + (n2 + 1) * 512],
                        start=(t == 0),
                        stop=(t == KI - 1),
                    )
            ysb = ysb_pool.tile([128, 1024], F32)
            nc.vector.tensor_scalar(
                ysb[:], yp[:], 1.0, -1.0, op0=ALU.min, op1=ALU.max
            )
            nc.sync.dma_start(
                y_d[b * 128:(b + 1) * 128, h * 1024:(h + 1) * 1024], ysb[:]
            )

    for c in reversed(ctxs):
        c.__exit__(None, None, None)


def build_program():
    nc = bacc.Bacc(
        "TRN2",
        target_bir_lowering=False,
        debug=False,
        num_devices=NCORES,
    )
    x_d = nc.dram_tensor("x", [BSH, IN], F32, kind="ExternalInput")
    w_d = nc.dram_tensor("weight", [OUT, IN], F32, kind="ExternalInput")
    gam_d = nc.dram_tensor("gamma_blk", [128, KI], F32, kind="ExternalInput")
    bet_d = nc.dram_tensor("beta_blk", [128, KI], F32, kind="ExternalInput")
    idf_d = nc.dram_tensor("ident_f16", [128, 128], F16, kind="ExternalInput")
    idb_d = nc.dram_tensor("ident_bf16", [128, 128], BF16, kind="ExternalInput")
    y_d = nc.dram_tensor("y", [BSH, OUT], F32, kind="ExternalOutput")

    with tile.TileContext(nc) as tc:
        build_kernel_body(
            tc, y_d[:, :], x_d[:, :], w_d[:, :], gam_d[:, :], bet_d[:, :],
            idf_d[:, :], idb_d[:, :],
        )
    nc.compile()
    return nc


_CACHE = {}


def _get_program():
    if "nc" not in _CACHE:
        _CACHE["nc"] = build_program()
    return _CACHE["nc"]


def make_in_maps(x, weight, gamma, beta):
    x = np.ascontiguousarray(np.asarray(x, dtype=np.float32))
    weight = np.ascontiguousarray(np.asarray(weight, dtype=np.float32))
    gamma = np.asarray(gamma, dtype=np.float32)
    beta = np.asarray(beta, dtype=np.float32)
    gamma_blk = np.ascontiguousarray(gamma.reshape(KI, 128).T)
    beta_blk = np.ascontiguousarray(beta.reshape(KI, 128).T)
    ident_f = np.eye(128, dtype=np.float16)
    ident_b = np.eye(128, dtype=ml_dtypes.bfloat16)
    in_maps = []
    for j in range(NCORES):
        in_maps.append({
            "x": np.ascontiguousarray(x[j * BSH:(j + 1) * BSH]),
            "weight": weight,
            "gamma_blk": gamma_blk,
            "beta_blk": beta_blk,
            "ident_f16": ident_f,
            "ident_bf16": ident_b,
        })
    return in_maps


def run(x, weight, gamma, beta, **spmd_kwargs):
    """Run on hardware; returns (y_full, BassKernelResults)."""
    nc = _get_program()
    in_maps = make_in_maps(x, weight, gamma, beta)
    res = run_bass_kernel_spmd(nc, in_maps, core_ids=list(range(NCORES)), **spmd_kwargs)
    y = np.concatenate([r["y"] for r in res.results], axis=0)
    return np.asarray(y, dtype=np.float32), res


def run_traced(x, weight, gamma, beta, profile_dir=None):
    """Run with NTFF capture via the axon sidechannel; returns
    (y_full, per_core_exec_ns, profile_dir)."""
    import ctypes, tempfile
    from concourse import bass2jax
    import gauge.profiler
    from concourse._compat import FishPath

    nc = _get_program()
    in_maps = make_in_maps(x, weight, gamma, beta)

    lib = ctypes.CDLL("/opt/axon/libaxon_pjrt.so")
    lib.axon_start_nrt_profile.argtypes = [
        ctypes.POINTER(ctypes.c_int64), ctypes.c_size_t]
    lib.axon_start_nrt_profile.restype = ctypes.c_int64
    lib.axon_stop_nrt_profile.argtypes = [ctypes.c_char_p]
    lib.axon_stop_nrt_profile.restype = ctypes.c_int64

    if profile_dir is None:
        profile_dir = tempfile.mkdtemp(prefix="ntff_")
    rc = lib.axon_start_nrt_profile(None, 0)
    assert rc == 0, f"axon_start_nrt_profile rc={rc}"
    try:
        results = bass2jax.run_bass_via_pjrt(nc, in_maps, n_cores=NCORES)
    finally:
        n = lib.axon_stop_nrt_profile(profile_dir.encode())
    y = np.concatenate([r["y"] for r in results], axis=0)
    if n <= 0:
        return np.asarray(y, dtype=np.float32), None, profile_dir

    profile = gauge.profiler.Profile(
        profile_path=FishPath(profile_dir),
        kernel_dev_mode=True,
        profile_on_exit=False,
        bass_kernel=nc.m,
        offline_processing=True,
        fname="*_body*",
    )
    perfetto_results = profile.to_perfetto(model_index=tuple(range(NCORES)))
    exec_ns = {}
    for i, pr in enumerate(perfetto_results or []):
        exec_ns[i] = pr.exec_time_ns
    return np.asarray(y, dtype=np.float32), exec_ns, profile_dir


def kernel(x, weight, gamma, beta):
    y, _ = run(x, weight, gamma, beta)
    return y
